# revision 1
# baseline (speedup 1.0000x reference)
"""Trainium2 Bass kernel for nn_LBLHighwayBiLmV2.

Computation (per batch element b, per layer l in {0,1}):
    fwd = band_fwd(fwd); bwd = band_bwd(bwd)          # 17-tap causal banded mix along L
    fwd = highway(fwd, fW[l], fb[l])  (2 steps)       # x = g*x + (1-g)*relu(nonlin)
    bwd = highway(bwd, bW[l], bb[l])
    if l: fwd += f_cache; bwd += b_cache
    out[l, b] = concat(fwd, bwd, axis=-1)

Sharding: data-parallel over batch, 1 sample per NeuronCore (8 cores).

Per-core dataflow (all fp32):
  - "home" layout for activations is transposed: vT = [H=512, L=2048] as 4 SBUF
    tiles [128, 2048] (feature chunks on partitions).
  - band: PE matmuls y^T[d,t] = sum_s x_nat[s,d] * S[s,t], with x natural
    (streamed from DRAM for layer 0, resident SBUF tiles for layer 1) as lhsT
    and precomputed banded coefficient blocks (host-built, tiny) as rhs.
  - highway matmuls: lhsT = W^T chunks (host-pre-transposed), rhs = vT -> PSUM,
    evacuated by ACT with fused bias+relu / bias+sigmoid.
  - gating on DVE: z = r + g*(v - r)  (3 tensor_tensor ops per chunk).
  - output transpose back to natural layout on PE (128x128 blocks into PSUM),
    evacuated ACT/DVE, DMA'd to DRAM.
"""

import os
import sys
from contextlib import ExitStack

import numpy as np

sys.path.insert(0, "/opt/trn_rl_repo")

import concourse.bacc as bacc
import concourse.bass as bass
import concourse.mybir as mybir
import concourse.tile as tile
from concourse.masks import make_identity

F32 = mybir.dt.float32

N_LAYERS = 2
N_HW = 2
B, L, H = 8, 2048, 512
WIDTH = 16
TWO_H = 2 * H
NT = L // 128      # 16 sequence tiles
ND = H // 128      # 4 feature chunks
NJ = L // 512      # 4 t-blocks of 512


# ----------------------------------------------------------------------------
# Host-side parameter prep (tiny, batch-independent)
# ----------------------------------------------------------------------------

def _fwd_blocks(w):
    """w: [17] logits (torch slicing convention). Returns (D_pad[128,1024],
    D_first[128,512]) fp32 blocks for the forward band matmuls.

    Forward score: S[s,t] = exp(w[16-(t-s)]) / Z_t for 0 <= t-s <= 16,
    Z_t = sum over valid d of exp(w[16-d]) (d = 0..min(16,t))."""
    w = np.asarray(w, np.float64)
    e = np.exp(w[16 - np.arange(17)] - w.max())      # e[d] = exp(w[16-d]) scaled
    cf = (e / e.sum()).astype(np.float64)            # full-column weights

    s = np.arange(128)[:, None]
    c = np.arange(1024)[None, :]
    d = (c - 384) - s
    Dp = np.where((d >= 0) & (d <= 16), cf[np.clip(d, 0, 16)], 0.0)

    t = np.arange(512)[None, :]
    d2 = t - s
    Zt = np.cumsum(e)[np.minimum(t, 16)]             # truncated norm for t<16
    Df = np.where((d2 >= 0) & (d2 <= 16), e[np.clip(d2, 0, 16)] / Zt, 0.0)
    return Dp.astype(np.float32), Df.astype(np.float32)


def _bwd_blocks(w):
    """Backward: S[s,t] = exp(w[s-t]) / Z_t for 0 <= s-t <= 16,
    Z_t truncated for t > L-17. Returns (E_pad[128,1024], E_last[128,512])."""
    w = np.asarray(w, np.float64)
    e = np.exp(w - w.max())                          # e[d] = exp(w[d]) scaled
    cb = (e / e.sum()).astype(np.float64)

    s = np.arange(128)[:, None]
    c = np.arange(1024)[None, :]
    d = s - (c - 512)
    Ep = np.where((d >= 0) & (d <= 16), cb[np.clip(d, 0, 16)], 0.0)

    sg = 1920 + s
    tg = 1536 + np.arange(512)[None, :]
    d2 = sg - tg
    lim = np.minimum(16, (L - 1) - tg)
    Zt = np.cumsum(e)[np.clip(lim, 0, 16)]
    El = np.where((d2 >= 0) & (d2 <= lim), e[np.clip(d2, 0, 16)] / Zt, 0.0)
    return Ep.astype(np.float32), El.astype(np.float32)


def _prep_params(f_scores, b_scores, fW, fb, bW, bb):
    bandf = np.zeros((N_LAYERS, 128, 1536), np.float32)
    bandb = np.zeros((N_LAYERS, 128, 1536), np.float32)
    for l in range(N_LAYERS):
        Dp, Df = _fwd_blocks(f_scores[l])
        bandf[l, :, :1024] = Dp
        bandf[l, :, 1024:] = Df
        Ep, El = _bwd_blocks(b_scores[l])
        bandb[l, :, :1024] = Ep
        bandb[l, :, 1024:] = El

    # wt[l, dir, h, d, o] = W[l?,dir,h][o, d]  (torch Linear weight transposed)
    # bias packed directly in SBUF layout: [128, (l d h oc)] with
    # bias[p, ((l*2+d)*2+h)*8+oc] = b[l,d,h][oc*128+p]
    wt = np.zeros((N_LAYERS, 2, N_HW, H, TWO_H), np.float32)
    bias = np.zeros((128, N_LAYERS * 2 * N_HW * 8), np.float32)
    for l in range(N_LAYERS):
        for d_, Wsrc, bsrc in ((0, fW, fb), (1, bW, bb)):
            for h in range(N_HW):
                wt[l, d_, h] = np.ascontiguousarray(np.asarray(Wsrc[l, h]).T)
                col0 = ((l * 2 + d_) * N_HW + h) * 8
                bias[:, col0:col0 + 8] = (
                    np.asarray(bsrc[l, h]).reshape(TWO_H // 128, 128).T)
    return bandf, bandb, wt, bias


# ----------------------------------------------------------------------------
# Bass kernel
# ----------------------------------------------------------------------------

def _fwd_members(j, Dp, Df):
    """s-tile index + rhs AP for each matmul of forward band t-block j."""
    out = []
    if j == 0:
        out.append((0, Df))
        for m in (1, 2, 3):
            out.append((m, Dp[:, 384 - 128 * m:896 - 128 * m]))
    else:
        out.append((4 * j - 1, Dp[:, 512:1024]))
        for m in (0, 1, 2, 3):
            out.append((4 * j + m, Dp[:, 384 - 128 * m:896 - 128 * m]))
    return out


def _bwd_members(j, Ep, El):
    out = []
    for m in (0, 1, 2, 3, 4):
        i = 4 * j + m
        if i >= NT:
            continue
        if j == NJ - 1 and m == 3:
            out.append((i, El))
        else:
            out.append((i, Ep[:, 512 - 128 * m:1024 - 128 * m]))
    return out


def build_nc():
    nc = bacc.Bacc("TRN2", target_bir_lowering=False, debug=False)

    x_ap = nc.dram_tensor("x", [L, H], F32, kind="ExternalInput").ap()
    wt_ap = nc.dram_tensor("wt", [N_LAYERS, 2, N_HW, H, TWO_H], F32,
                           kind="ExternalInput").ap()
    bias_ap = nc.dram_tensor("bias", [128, N_LAYERS * 2 * N_HW * 8], F32,
                             kind="ExternalInput").ap()
    bandf_ap = nc.dram_tensor("bandf", [N_LAYERS, 128, 1536], F32,
                              kind="ExternalInput").ap()
    bandb_ap = nc.dram_tensor("bandb", [N_LAYERS, 128, 1536], F32,
                              kind="ExternalInput").ap()
    out_ap = nc.dram_tensor("out", [N_LAYERS, L, TWO_H], F32,
                            kind="ExternalOutput").ap()

    with tile.TileContext(nc) as tc, ExitStack() as ctx:
        const_pool = ctx.enter_context(tc.tile_pool(name="const", bufs=1))
        band_pool = ctx.enter_context(tc.tile_pool(name="band", bufs=1))
        wt_pool = ctx.enter_context(tc.tile_pool(name="wtp", bufs=4))
        xin_pool = ctx.enter_context(tc.tile_pool(name="xin", bufs=3))
        act_pool = ctx.enter_context(tc.tile_pool(name="act", bufs=1))
        rg_pool = ctx.enter_context(tc.tile_pool(name="rg", bufs=2))
        znat_pool = ctx.enter_context(tc.tile_pool(name="znat", bufs=1))
        ztr_pool = ctx.enter_context(tc.tile_pool(name="ztr", bufs=2))
        mm_psum = ctx.enter_context(tc.tile_pool(name="mmp", bufs=5, space="PSUM"))
        tp_psum = ctx.enter_context(tc.tile_pool(name="tpp", bufs=3, space="PSUM"))

        identity = const_pool.tile([128, 128], F32, tag="ident")
        make_identity(nc, identity[:])
        bias_sb = const_pool.tile([128, N_LAYERS * 2 * N_HW * 8], F32, tag="bias")
        nc.sync.dma_start(bias_sb[:], bias_ap)

        # persistent activation tiles, one [128, 2048] per feature chunk
        A = [act_pool.tile([128, L], F32, tag=f"A{c}", name=f"A{c}")
             for c in range(ND)]
        Bt = [act_pool.tile([128, L], F32, tag=f"B{c}", name=f"B{c}")
              for c in range(ND)]
        C = [act_pool.tile([128, L], F32, tag=f"C{c}", name=f"C{c}")
             for c in range(ND)]
        znat = [znat_pool.tile([128, H], F32, tag=f"zn{k}", name=f"zn{k}")
                for k in range(NT)]

        def band(layer, dir_, dst):
            """dst: list of 4 chunk tiles [128, 2048] receiving y^T."""
            bsrc = bandf_ap if dir_ == 0 else bandb_ap
            bsb = band_pool.tile([128, 1536], F32, tag="bandblk")
            nc.sync.dma_start(bsb[:], bsrc[layer])
            Pad = bsb[:, 0:1024]
            Spc = bsb[:, 1024:1536]
            members = _fwd_members if dir_ == 0 else _bwd_members
            for j in range(NJ):
                mem = members(j, Pad, Spc)
                # gather lhsT sources for the s-tiles of this block
                srcs = {}
                for (i, _) in mem:
                    if layer == 0:
                        xt = xin_pool.tile([128, H], F32, tag="xstream")
                        nc.sync.dma_start(xt[:], x_ap[i * 128:(i + 1) * 128, :])
                        srcs[i] = xt
                    else:
                        srcs[i] = znat[i]
                for dc in range(ND):
                    ps = mm_psum.tile([128, 512], F32, tag="mm")
                    n = len(mem)
                    for k, (i, rhs) in enumerate(mem):
                        nc.tensor.matmul(
                            ps[:],
                            srcs[i][:, dc * 128:(dc + 1) * 128],
                            rhs,
                            start=(k == 0),
                            stop=(k == n - 1),
                        )
                    dstap = dst[dc][:, j * 512:(j + 1) * 512]
                    if dc % 2 == 0:
                        nc.scalar.copy(dstap, ps[:])
                    else:
                        nc.vector.tensor_copy(dstap, ps[:])

        def highway(layer, dir_, h, v, dst, resid=None):
            """v: input chunk tiles (transposed layout); dst: output chunk tiles.
            z = r + g*(v - r), r = relu(nl + bn), g = sigmoid(gt + bg)."""
            wts = []
            for dc in range(ND):
                wtile = wt_pool.tile([128, TWO_H], F32, tag="wt")
                nc.sync.dma_start(
                    wtile[:], wt_ap[layer, dir_, h, dc * 128:(dc + 1) * 128, :])
                wts.append(wtile)
            bcol0 = ((layer * 2 + dir_) * N_HW + h) * 8
            for c in range(ND):
                r = rg_pool.tile([128, L], F32, tag="r")
                g = rg_pool.tile([128, L], F32, tag="g")
                for part, oc in ((0, c), (1, 4 + c)):
                    for j in range(NJ):
                        ps = mm_psum.tile([128, 512], F32, tag="mm")
                        for dc in range(ND):
                            nc.tensor.matmul(
                                ps[:],
                                wts[dc][:, oc * 128:(oc + 1) * 128],
                                v[dc][:, j * 512:(j + 1) * 512],
                                start=(dc == 0),
                                stop=(dc == ND - 1),
                            )
                        bap = bias_sb[:, bcol0 + oc:bcol0 + oc + 1]
                        tgt = (r if part == 0 else g)[:, j * 512:(j + 1) * 512]
                        func = (mybir.ActivationFunctionType.Relu if part == 0
                                else mybir.ActivationFunctionType.Sigmoid)
                        nc.scalar.activation(tgt, ps[:], func, bias=bap)
                # gating for chunk c on DVE (two half-rows to limit tile size)
                for hb in range(2):
                    sl = slice(hb * 1024, (hb + 1) * 1024)
                    tmp = rg_pool.tile([128, 1024], F32, tag="tmp")
                    nc.vector.tensor_sub(tmp[:], v[c][:, sl], r[:, sl])
                    nc.vector.tensor_mul(tmp[:], tmp[:], g[:, sl])
                    nc.vector.tensor_add(dst[c][:, sl], tmp[:], r[:, sl])
                    if resid is not None:
                        nc.vector.tensor_add(
                            dst[c][:, sl], dst[c][:, sl], resid[c][:, sl])

        def transpose_out(layer, dir_, src, keep_nat):
            """src: 4 chunk tiles [128,2048] (transposed). Writes natural-layout
            DRAM out[layer, :, dir_*H:(dir_+1)*H]; optionally keeps natural
            tiles in SBUF (layer-0, as next layer's band input)."""
            for k in range(NT):
                ps = tp_psum.tile([128, 512], F32, tag="tp")
                for dc in range(ND):
                    nc.tensor.transpose(
                        ps[:, dc * 128:(dc + 1) * 128],
                        src[dc][:, k * 128:(k + 1) * 128],
                        identity[:],
                    )
                if keep_nat:
                    zt = znat[k]
                else:
                    zt = ztr_pool.tile([128, H], F32, tag="ztr")
                if k % 2 == 0:
                    nc.scalar.copy(zt[:], ps[:])
                else:
                    nc.vector.tensor_copy(zt[:], ps[:])
                nc.sync.dma_start(
                    out_ap[layer, k * 128:(k + 1) * 128,
                           dir_ * H:(dir_ + 1) * H],
                    zt[:],
                )

        for dir_ in (0, 1):
            # layer 0
            band(0, dir_, A)
            highway(0, dir_, 0, A, Bt)
            highway(0, dir_, 1, Bt, C)
            transpose_out(0, dir_, C, keep_nat=True)
            # layer 1 (band reads resident natural tiles; residual adds C)
            band(1, dir_, A)
            highway(1, dir_, 0, A, Bt)
            highway(1, dir_, 1, Bt, A, resid=C)
            transpose_out(1, dir_, A, keep_nat=False)

    nc.compile()
    return nc


_NC_CACHE = None
LAST_RESULTS = None


def _get_nc():
    global _NC_CACHE
    if _NC_CACHE is None:
        _NC_CACHE = build_nc()
    return _NC_CACHE


def kernel(inputs, masks, f_scores, b_scores, fW, fb, bW, bb):
    global LAST_RESULTS
    from concourse.bass_utils import run_bass_kernel_spmd

    inputs = np.asarray(inputs, np.float32)
    bandf, bandb, wt, bias = _prep_params(
        np.asarray(f_scores), np.asarray(b_scores),
        np.asarray(fW), np.asarray(fb), np.asarray(bW), np.asarray(bb))

    nc = _get_nc()
    in_maps = [
        {"x": np.ascontiguousarray(inputs[b]), "wt": wt, "bias": bias,
         "bandf": bandf, "bandb": bandb}
        for b in range(B)
    ]
    res = run_bass_kernel_spmd(nc, in_maps, core_ids=list(range(B)),
                               trace=bool(os.environ.get("BASS_TRACE")))
    LAST_RESULTS = res
    out = np.stack([res.results[b]["out"] for b in range(B)], axis=1)
    return out



# revision 3
# speedup vs baseline: 1.3121x; 1.3121x over previous
"""Trainium2 Bass kernel for nn_LBLHighwayBiLmV2 (bf16 compute).

Computation (per batch element b, per layer l in {0,1}):
    fwd = band_fwd(fwd); bwd = band_bwd(bwd)          # 17-tap causal banded mix along L
    fwd = highway(fwd, fW[l], fb[l])  (2 steps)       # x = g*x + (1-g)*relu(nonlin)
    bwd = highway(bwd, bW[l], bb[l])
    if l: fwd += f_cache; bwd += b_cache
    out[l, b] = concat(fwd, bwd, axis=-1)

Sharding: data-parallel over batch, 1 sample per NeuronCore (8 cores).

Per-core dataflow (bf16 operands, fp32 PSUM accumulate):
  - "home" layout for activations is transposed: vT = [H=512, L=2048] as 4 SBUF
    tiles [128, 2048] (feature chunks on partitions).
  - band: PE matmuls y^T[d,t] = sum_s x_nat[s,d] * S[s,t], with x natural
    (streamed from DRAM for layer 0, resident SBUF tiles for layer 1) as lhsT
    and precomputed banded coefficient blocks (host-built, tiny) as rhs.
  - highway matmuls: lhsT = W^T chunks (host-pre-transposed), rhs = vT -> PSUM;
    gate half evacuated by ACT with fused bias+sigmoid, nonlinear half by DVE
    tensor_scalar with fused bias-add + max(0).
  - gating on DVE: z = r + g*(v - r)  (3 tensor_tensor ops per chunk).
  - output transpose back to natural layout on PE (128x128 blocks into PSUM),
    evacuated ACT/DVE, DMA'd to DRAM as bf16 (host casts to fp32).
"""

import os
import sys
from contextlib import ExitStack

import numpy as np

sys.path.insert(0, "/opt/trn_rl_repo")

import concourse.bacc as bacc
import concourse.bass as bass
import concourse.mybir as mybir
import concourse.tile as tile
from concourse.masks import make_identity

F32 = mybir.dt.float32
BF16 = mybir.dt.bfloat16
NP_BF16 = mybir.dt.np(BF16)

N_LAYERS = 2
N_HW = 2
B, L, H = 8, 2048, 512
WIDTH = 16
TWO_H = 2 * H
NT = L // 128      # 16 sequence tiles
ND = H // 128      # 4 feature chunks
NJ = L // 512      # 4 t-blocks of 512


# ----------------------------------------------------------------------------
# Host-side parameter prep (tiny, batch-independent)
# ----------------------------------------------------------------------------

def _fwd_blocks(w):
    """w: [17] logits (torch slicing convention). Returns (D_pad[128,1024],
    D_first[128,512]) fp32 blocks for the forward band matmuls.

    Forward score: S[s,t] = exp(w[16-(t-s)]) / Z_t for 0 <= t-s <= 16,
    Z_t = sum over valid d of exp(w[16-d]) (d = 0..min(16,t))."""
    w = np.asarray(w, np.float64)
    e = np.exp(w[16 - np.arange(17)] - w.max())      # e[d] = exp(w[16-d]) scaled
    cf = (e / e.sum()).astype(np.float64)            # full-column weights

    s = np.arange(128)[:, None]
    c = np.arange(1024)[None, :]
    d = (c - 384) - s
    Dp = np.where((d >= 0) & (d <= 16), cf[np.clip(d, 0, 16)], 0.0)

    t = np.arange(512)[None, :]
    d2 = t - s
    Zt = np.cumsum(e)[np.minimum(t, 16)]             # truncated norm for t<16
    Df = np.where((d2 >= 0) & (d2 <= 16), e[np.clip(d2, 0, 16)] / Zt, 0.0)
    return Dp.astype(np.float32), Df.astype(np.float32)


def _bwd_blocks(w):
    """Backward: S[s,t] = exp(w[s-t]) / Z_t for 0 <= s-t <= 16,
    Z_t truncated for t > L-17. Returns (E_pad[128,1024], E_last[128,512])."""
    w = np.asarray(w, np.float64)
    e = np.exp(w - w.max())                          # e[d] = exp(w[d]) scaled
    cb = (e / e.sum()).astype(np.float64)

    s = np.arange(128)[:, None]
    c = np.arange(1024)[None, :]
    d = s - (c - 512)
    Ep = np.where((d >= 0) & (d <= 16), cb[np.clip(d, 0, 16)], 0.0)

    sg = 1920 + s
    tg = 1536 + np.arange(512)[None, :]
    d2 = sg - tg
    lim = np.minimum(16, (L - 1) - tg)
    Zt = np.cumsum(e)[np.clip(lim, 0, 16)]
    El = np.where((d2 >= 0) & (d2 <= lim), e[np.clip(d2, 0, 16)] / Zt, 0.0)
    return Ep.astype(np.float32), El.astype(np.float32)


def _prep_params(f_scores, b_scores, fW, fb, bW, bb):
    bandf = np.zeros((N_LAYERS, 128, 1536), np.float32)
    bandb = np.zeros((N_LAYERS, 128, 1536), np.float32)
    for l in range(N_LAYERS):
        Dp, Df = _fwd_blocks(f_scores[l])
        bandf[l, :, :1024] = Dp
        bandf[l, :, 1024:] = Df
        Ep, El = _bwd_blocks(b_scores[l])
        bandb[l, :, :1024] = Ep
        bandb[l, :, 1024:] = El

    # wt[l, dir, h, d, o] = W[l?,dir,h][o, d]  (torch Linear weight transposed)
    # bias packed directly in SBUF layout: [128, (l d h oc)] with
    # bias[p, ((l*2+d)*2+h)*8+oc] = b[l,d,h][oc*128+p]
    wt = np.zeros((N_LAYERS, 2, N_HW, H, TWO_H), np.float32)
    bias = np.zeros((128, N_LAYERS * 2 * N_HW * 8), np.float32)
    for l in range(N_LAYERS):
        for d_, Wsrc, bsrc in ((0, fW, fb), (1, bW, bb)):
            for h in range(N_HW):
                wt[l, d_, h] = np.ascontiguousarray(np.asarray(Wsrc[l, h]).T)
                col0 = ((l * 2 + d_) * N_HW + h) * 8
                bias[:, col0:col0 + 8] = (
                    np.asarray(bsrc[l, h]).reshape(TWO_H // 128, 128).T)
    return (bandf.astype(NP_BF16), bandb.astype(NP_BF16),
            wt.astype(NP_BF16), bias)


def build_in_maps(inputs, f_scores, b_scores, fW, fb, bW, bb):
    """Per-core input dicts for run_bass_kernel_spmd (bf16 payloads)."""
    bandf, bandb, wt, bias = _prep_params(
        np.asarray(f_scores), np.asarray(b_scores),
        np.asarray(fW), np.asarray(fb), np.asarray(bW), np.asarray(bb))
    x16 = np.ascontiguousarray(np.asarray(inputs, np.float32)).astype(NP_BF16)
    return [
        {"x": x16[b], "wt": wt, "bias": bias, "bandf": bandf, "bandb": bandb}
        for b in range(B)
    ]


# ----------------------------------------------------------------------------
# Bass kernel
# ----------------------------------------------------------------------------

def _fwd_members(j, Dp, Df):
    """s-tile index + rhs AP for each matmul of forward band t-block j."""
    out = []
    if j == 0:
        out.append((0, Df))
        for m in (1, 2, 3):
            out.append((m, Dp[:, 384 - 128 * m:896 - 128 * m]))
    else:
        out.append((4 * j - 1, Dp[:, 512:1024]))
        for m in (0, 1, 2, 3):
            out.append((4 * j + m, Dp[:, 384 - 128 * m:896 - 128 * m]))
    return out


def _bwd_members(j, Ep, El):
    out = []
    for m in (0, 1, 2, 3, 4):
        i = 4 * j + m
        if i >= NT:
            continue
        if j == NJ - 1 and m == 3:
            out.append((i, El))
        else:
            out.append((i, Ep[:, 512 - 128 * m:1024 - 128 * m]))
    return out


def build_nc():
    nc = bacc.Bacc("TRN2", target_bir_lowering=False, debug=False)

    x_ap = nc.dram_tensor("x", [L, H], BF16, kind="ExternalInput").ap()
    wt_ap = nc.dram_tensor("wt", [N_LAYERS, 2, N_HW, H, TWO_H], BF16,
                           kind="ExternalInput").ap()
    bias_ap = nc.dram_tensor("bias", [128, N_LAYERS * 2 * N_HW * 8], F32,
                             kind="ExternalInput").ap()
    bandf_ap = nc.dram_tensor("bandf", [N_LAYERS, 128, 1536], BF16,
                              kind="ExternalInput").ap()
    bandb_ap = nc.dram_tensor("bandb", [N_LAYERS, 128, 1536], BF16,
                              kind="ExternalInput").ap()
    out_ap = nc.dram_tensor("out", [N_LAYERS, L, TWO_H], BF16,
                            kind="ExternalOutput").ap()

    with tile.TileContext(nc) as tc, ExitStack() as ctx:
        const_pool = ctx.enter_context(tc.tile_pool(name="const", bufs=1))
        band_pool = ctx.enter_context(tc.tile_pool(name="band", bufs=1))
        wt_pool = ctx.enter_context(tc.tile_pool(name="wtp", bufs=4))
        xin_pool = ctx.enter_context(tc.tile_pool(name="xin", bufs=3))
        act_pool = ctx.enter_context(tc.tile_pool(name="act", bufs=1))
        rg_pool = ctx.enter_context(tc.tile_pool(name="rg", bufs=2))
        znat_pool = ctx.enter_context(tc.tile_pool(name="znat", bufs=1))
        ztr_pool = ctx.enter_context(tc.tile_pool(name="ztr", bufs=2))
        mm_psum = ctx.enter_context(tc.tile_pool(name="mmp", bufs=5, space="PSUM"))
        tp_psum = ctx.enter_context(tc.tile_pool(name="tpp", bufs=3, space="PSUM"))

        identity = const_pool.tile([128, 128], BF16, tag="ident")
        make_identity(nc, identity[:])
        bias_sb = const_pool.tile([128, N_LAYERS * 2 * N_HW * 8], F32, tag="bias")
        nc.sync.dma_start(bias_sb[:], bias_ap)

        # persistent activation tiles, one [128, 2048] per feature chunk
        A = [act_pool.tile([128, L], BF16, tag=f"A{c}", name=f"A{c}")
             for c in range(ND)]
        Bt = [act_pool.tile([128, L], BF16, tag=f"B{c}", name=f"B{c}")
              for c in range(ND)]
        C = [act_pool.tile([128, L], BF16, tag=f"C{c}", name=f"C{c}")
             for c in range(ND)]
        znat = [znat_pool.tile([128, H], BF16, tag=f"zn{k}", name=f"zn{k}")
                for k in range(NT)]

        def band(layer, dir_, dst):
            """dst: list of 4 chunk tiles [128, 2048] receiving y^T."""
            bsrc = bandf_ap if dir_ == 0 else bandb_ap
            bsb = band_pool.tile([128, 1536], BF16, tag="bandblk")
            nc.sync.dma_start(bsb[:], bsrc[layer])
            Pad = bsb[:, 0:1024]
            Spc = bsb[:, 1024:1536]
            members = _fwd_members if dir_ == 0 else _bwd_members
            for j in range(NJ):
                mem = members(j, Pad, Spc)
                # gather lhsT sources for the s-tiles of this block
                srcs = {}
                for (i, _) in mem:
                    if layer == 0:
                        xt = xin_pool.tile([128, H], BF16, tag="xstream")
                        nc.sync.dma_start(xt[:], x_ap[i * 128:(i + 1) * 128, :])
                        srcs[i] = xt
                    else:
                        srcs[i] = znat[i]
                for dc in range(ND):
                    ps = mm_psum.tile([128, 512], F32, tag="mm")
                    n = len(mem)
                    for k, (i, rhs) in enumerate(mem):
                        nc.tensor.matmul(
                            ps[:],
                            srcs[i][:, dc * 128:(dc + 1) * 128],
                            rhs,
                            start=(k == 0),
                            stop=(k == n - 1),
                        )
                    dstap = dst[dc][:, j * 512:(j + 1) * 512]
                    if dc % 2 == 0:
                        nc.scalar.copy(dstap, ps[:])
                    else:
                        nc.vector.tensor_copy(dstap, ps[:])

        def highway(layer, dir_, h, v, dst, resid=None):
            """v: input chunk tiles (transposed layout); dst: output chunk tiles.
            z = r + g*(v - r), r = relu(nl + bn), g = sigmoid(gt + bg)."""
            wts = []
            for dc in range(ND):
                wtile = wt_pool.tile([128, TWO_H], BF16, tag="wt")
                nc.sync.dma_start(
                    wtile[:], wt_ap[layer, dir_, h, dc * 128:(dc + 1) * 128, :])
                wts.append(wtile)
            bcol0 = ((layer * 2 + dir_) * N_HW + h) * 8
            for c in range(ND):
                r = rg_pool.tile([128, L], BF16, tag="r")
                g = rg_pool.tile([128, L], BF16, tag="g")
                for part, oc in ((0, c), (1, 4 + c)):
                    for j in range(NJ):
                        ps = mm_psum.tile([128, 512], F32, tag="mm")
                        for dc in range(ND):
                            nc.tensor.matmul(
                                ps[:],
                                wts[dc][:, oc * 128:(oc + 1) * 128],
                                v[dc][:, j * 512:(j + 1) * 512],
                                start=(dc == 0),
                                stop=(dc == ND - 1),
                            )
                        bap = bias_sb[:, bcol0 + oc:bcol0 + oc + 1]
                        tgt = (r if part == 0 else g)[:, j * 512:(j + 1) * 512]
                        if part == 0:
                            # r = max(psum + bias, 0) fused on DVE
                            nc.vector.tensor_scalar(
                                tgt, ps[:], bap, 0.0,
                                mybir.AluOpType.add, mybir.AluOpType.max)
                        else:
                            nc.scalar.activation(
                                tgt, ps[:],
                                mybir.ActivationFunctionType.Sigmoid, bias=bap)
                # gating for chunk c on DVE (two half-rows to limit tile size)
                for hb in range(2):
                    sl = slice(hb * 1024, (hb + 1) * 1024)
                    tmp = rg_pool.tile([128, 1024], BF16, tag="tmp")
                    nc.vector.tensor_sub(tmp[:], v[c][:, sl], r[:, sl])
                    nc.vector.tensor_mul(tmp[:], tmp[:], g[:, sl])
                    nc.vector.tensor_add(dst[c][:, sl], tmp[:], r[:, sl])
                    if resid is not None:
                        nc.vector.tensor_add(
                            dst[c][:, sl], dst[c][:, sl], resid[c][:, sl])

        def transpose_out(layer, dir_, src, keep_nat):
            """src: 4 chunk tiles [128,2048] (transposed). Writes natural-layout
            DRAM out[layer, :, dir_*H:(dir_+1)*H]; optionally keeps natural
            tiles in SBUF (layer-0, as next layer's band input)."""
            for k in range(NT):
                ps = tp_psum.tile([128, 512], BF16, tag="tp")
                for dc in range(ND):
                    nc.tensor.transpose(
                        ps[:, dc * 128:(dc + 1) * 128],
                        src[dc][:, k * 128:(k + 1) * 128],
                        identity[:],
                    )
                if keep_nat:
                    zt = znat[k]
                else:
                    zt = ztr_pool.tile([128, H], BF16, tag="ztr")
                if k % 2 == 0:
                    nc.scalar.copy(zt[:], ps[:])
                else:
                    nc.vector.tensor_copy(zt[:], ps[:])
                nc.sync.dma_start(
                    out_ap[layer, k * 128:(k + 1) * 128,
                           dir_ * H:(dir_ + 1) * H],
                    zt[:],
                )

        for dir_ in (0, 1):
            # layer 0
            band(0, dir_, A)
            highway(0, dir_, 0, A, Bt)
            highway(0, dir_, 1, Bt, C)
            transpose_out(0, dir_, C, keep_nat=True)
            # layer 1 (band reads resident natural tiles; residual adds C)
            band(1, dir_, A)
            highway(1, dir_, 0, A, Bt)
            highway(1, dir_, 1, Bt, A, resid=C)
            transpose_out(1, dir_, A, keep_nat=False)

    nc.compile()
    return nc


_NC_CACHE = None
LAST_RESULTS = None


def _get_nc():
    global _NC_CACHE
    if _NC_CACHE is None:
        _NC_CACHE = build_nc()
    return _NC_CACHE


def kernel(inputs, masks, f_scores, b_scores, fW, fb, bW, bb):
    global LAST_RESULTS
    from concourse.bass_utils import run_bass_kernel_spmd

    in_maps = build_in_maps(inputs, f_scores, b_scores, fW, fb, bW, bb)
    nc = _get_nc()
    res = run_bass_kernel_spmd(nc, in_maps, core_ids=list(range(B)),
                               trace=bool(os.environ.get("BASS_TRACE")))
    LAST_RESULTS = res
    out = np.stack([res.results[b]["out"].astype(np.float32) for b in range(B)],
                   axis=1)
    return out
